# revision 1
# baseline (speedup 1.0000x reference)
"""MetaConv1d Trainium2 kernel.

Per-sample hypernetwork-generated conv1d:
  w1 = meta @ W1.T + b1            (BN, 64, 32)
  H  = w1 @ W2.T + b2              (BN, 64, 192)   [192 = (o=64, j=3) interleaved 3o+j]
  b  = meta @ BL.T + bb            (BN, 64)
  out[n,t,o] = sum_{c,j} H[n,c,3o+j] * x[n,c,t+j] + b[n,o]

Sharding: batch*node dim (6624) split evenly over 8 cores (828 each).
All matmuls run in bf16 with fp32 PSUM accumulation.

Device-side dataflow per core (per n-tile of <=128 samples):
  step1 (batched):  psum = metaT_aug.T @ w1aug  -> W1out (nt, 2048) bf16
  B (batched):      psum = blaug.T @ metaT_aug  -> B_sb (64, nt) fp32  (bias columns)
  bulk transpose:   64 PE transposes (nt,32)->(32,nt) -> W1T (33, 64*128) bf16
                    (row 32 = ones so step2 picks up w2 bias row)
  per sample:
    PE transpose x-slice (128,64)->(64,128) -> xT bf16
    step2: matmul(lhsT=W1T[:, n::128] (33,64), rhs=w2aug (33,192)) -> H (64,192) bf16
    conv:  3 accum matmuls lhsT=H[:, j::3] (64,64), rhs=xT[:, j:j+126] -> psum (64,126)
    bias:  DVE tensor_scalar_add(outT, psum, B_sb[:, n])  (per-partition bias) fp32
    PE transpose outT (64,126)->(126,64) fp32 -> final evac -> batched DMA out
"""

import numpy as np
import ml_dtypes

import concourse.mybir as mybir
import concourse.bacc as bacc
from concourse.tile import TileContext
from concourse.bass_utils import run_bass_kernel_spmd

BF16 = mybir.dt.bfloat16
F32 = mybir.dt.float32

B = 32
N = 207
BN = B * N            # 6624
L = 128
C = 64                # in channels
O = 64                # out channels
KK = 3
META = 32
LOUT = L - KK + 1     # 126
NCORES = 8
PER = BN // NCORES    # 828
NTS = 128             # n-tile stride (samples per tile)
GX = 16               # samples per x-load DMA
GO = 8                # samples per out-store DMA


def build_program(per=PER):
    """Build the per-core Bass program (identical on all 8 cores)."""
    nc = bacc.Bacc("TRN2", target_bir_lowering=False)

    x_d = nc.dram_tensor("x", (per, L, C), F32, kind="ExternalInput")
    metaT_d = nc.dram_tensor("metaT", (META + 1, per), BF16, kind="ExternalInput")
    w1aug_d = nc.dram_tensor("w1aug", (META + 1, C * META), BF16, kind="ExternalInput")
    w2aug_d = nc.dram_tensor("w2aug", (META + 1, O * KK), BF16, kind="ExternalInput")
    blaug_d = nc.dram_tensor("blaug", (META + 1, O), BF16, kind="ExternalInput")
    identB_d = nc.dram_tensor("identB", (128, 128), BF16, kind="ExternalInput")
    identF_d = nc.dram_tensor("identF", (64, 64), F32, kind="ExternalInput")
    out_d = nc.dram_tensor("out", (per, LOUT, O), F32, kind="ExternalOutput")

    n_tiles = [(t, min(NTS, per - t)) for t in range(0, per, NTS)]

    with TileContext(nc) as tc:
        with (
            tc.tile_pool(name="const", bufs=1) as cpool,
            tc.tile_pool(name="wpool", bufs=2) as wpool,
            tc.tile_pool(name="xpool", bufs=3) as xpool,
            tc.tile_pool(name="spool", bufs=4) as spool,
            tc.tile_pool(name="opool", bufs=3) as opool,
            tc.tile_pool(name="pspool", bufs=2, space="PSUM") as pspool,
        ):
            # ---- constants (loaded once) ----
            w1aug = cpool.tile([META + 1, C * META], BF16)
            nc.sync.dma_start(w1aug[:, :], w1aug_d[:, :])
            w2aug = cpool.tile([META + 1, O * KK], BF16)
            nc.sync.dma_start(w2aug[:, :], w2aug_d[:, :])
            blaug = cpool.tile([META + 1, O], BF16)
            nc.sync.dma_start(blaug[:, :], blaug_d[:, :])
            identB = cpool.tile([128, 128], BF16)
            nc.sync.dma_start(identB[:, :], identB_d[:, :])
            identF = cpool.tile([64, 64], F32)
            nc.sync.dma_start(identF[:, :], identF_d[:, :])

            for n0, nt in n_tiles:
                # ---- per-tile batched hypernet stage ----
                metaT_sb = wpool.tile([META + 1, nt], BF16, tag="metaT")
                nc.sync.dma_start(metaT_sb[:, :], metaT_d[:, n0 : n0 + nt])

                # W1out holds 33 columns per c-chunk: 32 e-values + a ones column
                # (the ones column transposes into the ones row of W1T, which
                # multiplies the bias row of w2aug in step2).
                W1out = wpool.tile(
                    [nt, C * (META + 1)], BF16, tag="w1out", padded_shape=[NTS, C * (META + 1)]
                )
                W1out_r = W1out[:, :].rearrange("p (c e) -> p c e", e=META + 1)
                nc.vector.memset(W1out_r[:, :, META : META + 1], 1.0)
                for k in range(4):
                    ps1 = pspool.tile([nt, 512], F32, tag="psBig", padded_shape=[NTS, 512])
                    nc.tensor.matmul(
                        ps1[:, :],
                        metaT_sb[:, :],
                        w1aug[:, k * 512 : (k + 1) * 512],
                        start=True,
                        stop=True,
                    )
                    nc.vector.tensor_copy(
                        W1out_r[:, k * 16 : (k + 1) * 16, 0:META],
                        ps1[:, :].rearrange("p (c e) -> p c e", e=META),
                    )

                psB = pspool.tile([O, nt], F32, tag="psC", padded_shape=[O, NTS])
                nc.tensor.matmul(psB[:, :], blaug[:, :], metaT_sb[:, :], start=True, stop=True)
                B_sb = wpool.tile([O, nt], F32, tag="Bsb", padded_shape=[O, NTS])
                nc.vector.tensor_copy(B_sb[:, :], psB[:, :])

                # ---- bulk transpose W1out -> W1T (33, 64*NTS) ----
                W1T = wpool.tile([META + 1, C * NTS], BF16, tag="w1t")
                for cc in range(C):
                    psT = pspool.tile(
                        [META + 1, nt], BF16, tag="psT", padded_shape=[META + 1, NTS]
                    )
                    nc.tensor.transpose(
                        psT[:, :],
                        W1out_r[0:nt, cc, :],
                        identB[0:nt, 0:nt],
                    )
                    nc.scalar.copy(W1T[:, cc * NTS : cc * NTS + nt], psT[:, :])

                W1T_r = W1T[:, :].rearrange("p (c n) -> p n c", n=NTS)

                # ---- per-sample stage ----
                for g0 in range(0, nt, GX):
                    gx = min(GX, nt - g0)
                    x_sb = xpool.tile([L, C * GX], BF16, tag="xsb")
                    nc.gpsimd.dma_start(
                        x_sb[:, 0 : C * gx].rearrange("l (g c) -> l g c", c=C),
                        x_d[n0 + g0 : n0 + g0 + gx, :, :].rearrange("g l c -> l g c"),
                    )
                    for o0 in range(g0, g0 + gx, GO):
                        go = min(GO, g0 + gx - o0)
                        out_sb = opool.tile([LOUT, O * GO], F32, tag="osb")
                        for ns in range(o0, o0 + go):
                            xo = (ns - g0) * C
                            # x transpose: (128, 64) -> (64, 128)
                            psX = pspool.tile([C, L], BF16, tag="psT")
                            nc.tensor.transpose(
                                psX[:, :], x_sb[:, xo : xo + C], identB[:, :]
                            )
                            xT = spool.tile([C, L], BF16, tag="xT")
                            nc.vector.tensor_copy(xT[:, :], psX[:, :])
                            # step2: H = W1c.T @ w2aug
                            psH = pspool.tile([C, O * KK], F32, tag="psBig")
                            nc.tensor.matmul(
                                psH[:, :], W1T_r[:, ns, :], w2aug[:, :],
                                start=True, stop=True,
                            )
                            H = spool.tile([C, O * KK], BF16, tag="H")
                            nc.scalar.copy(H[:, :], psH[:, :])
                            H_r = H[:, :].rearrange("p (o j) -> p j o", j=KK)
                            # conv: 3 accumulating matmuls into one psum bank
                            psC = pspool.tile([O, LOUT], F32, tag="psC")
                            for j in range(KK):
                                nc.tensor.matmul(
                                    psC[:, :],
                                    H_r[:, j, :],
                                    xT[:, j : j + LOUT],
                                    start=(j == 0),
                                    stop=(j == KK - 1),
                                )
                            # bias add (per-partition scalar = per-o), fp32 out
                            outT = spool.tile([O, LOUT], F32, tag="outT")
                            nc.vector.tensor_scalar_add(
                                outT[:, :], psC[:, :], B_sb[:, ns : ns + 1]
                            )
                            # output transpose: (64, 126) -> (126, 64)
                            psO = pspool.tile([LOUT, O], F32, tag="psO")
                            nc.tensor.transpose(psO[:, :], outT[:, :], identF[:, :])
                            oo = (ns - o0) * O
                            nc.scalar.copy(out_sb[:, oo : oo + O], psO[:, :])
                        nc.sync.dma_start(
                            out_d[n0 + o0 : n0 + o0 + go, :, :].rearrange(
                                "g t o -> t g o"
                            ),
                            out_sb[:, 0 : go * O].rearrange("t (g o) -> t g o", o=O),
                        )
    if not nc.is_finalized():
        nc.finalize()
    return nc


def _prep_consts(w1_w, w1_b, w2_w, w2_b, bl_w, bl_b, meta):
    bf = ml_dtypes.bfloat16
    w1aug = np.concatenate([w1_w.T, w1_b[None, :]], axis=0).astype(bf)
    w2aug = np.concatenate([w2_w.T, w2_b[None, :]], axis=0).astype(bf)
    blaug = np.concatenate([bl_w.T, bl_b[None, :]], axis=0).astype(bf)
    metaT = np.concatenate(
        [meta.T, np.ones((1, meta.shape[0]), np.float32)], axis=0
    ).astype(bf)
    identB = np.eye(128, dtype=bf)
    identF = np.eye(64, dtype=np.float32)
    return w1aug, w2aug, blaug, metaT, identB, identF


LAST_EXEC_NS = None
_NC_CACHE = {}


def kernel(meta_knowledge, input, w1_w, w1_b, w2_w, w2_b, bl_w, bl_b):
    global LAST_EXEC_NS
    import os

    w1aug, w2aug, blaug, metaT, identB, identF = _prep_consts(
        w1_w, w1_b, w2_w, w2_b, bl_w, bl_b, meta_knowledge
    )
    x_all = np.ascontiguousarray(input.reshape(BN, L, C), dtype=np.float32)

    if PER not in _NC_CACHE:
        _NC_CACHE[PER] = build_program(PER)
    nc = _NC_CACHE[PER]
    in_maps = []
    for i in range(NCORES):
        s = slice(i * PER, (i + 1) * PER)
        in_maps.append(
            {
                "x": np.ascontiguousarray(x_all[s]),
                "metaT": np.ascontiguousarray(metaT[:, s]),
                "w1aug": w1aug,
                "w2aug": w2aug,
                "blaug": blaug,
                "identB": identB,
                "identF": identF,
            }
        )
    trace = os.environ.get("KM_TRACE", "0") == "1"
    res = run_bass_kernel_spmd(
        nc, in_maps, core_ids=list(range(NCORES)), trace=trace
    )
    if res.exec_time_ns is not None:
        LAST_EXEC_NS = res.exec_time_ns
    out = np.concatenate([r["out"] for r in res.results], axis=0)
    return out.reshape(B, N, LOUT, O)



# revision 5
# speedup vs baseline: 3.7135x; 3.7135x over previous
"""MetaConv1d Trainium2 kernel (v2 — transpose-free, tile-position paired).

Per-sample hypernetwork-generated conv1d:
  W1 = meta @ w1.T + b1            (BN, 64, 33)   [aug: col 32 generates ones row]
  H  = W1 @ w2.T + b2              (BN, 64, 192)  [192 = (o, j) = 3o+j]
  b  = meta @ bl.T + bb            (BN, 64)
  out[n,t,o] = sum_{c,j} H[n,c,3o+j] * x[n,c,t+j] + b[n,o]

Sharding: batch*node dim (6624) split evenly over 8 cores (828 each).

Device dataflow (per n-tile of <=128 samples):
  step1: per channel c: psum = w1T3_c.T @ metaT  -> W1T (33, 64*128) bf16
         (c even -> PE tile (0,0), c odd -> (0,64); no transposes needed
          because out partitions = e come straight from the matmul)
  bias:  2 MMs (even/odd samples) -> B2 (128, npairs) f32
  per pair of samples (A,B):
    step2: MM_A (tile 0,0) -> psH[0:64], MM_B (tile 0,64) -> psH[64:128]
    H2 evac: one scalar copy (128, 192) -> bf16
    conv: 3 accumulating MMs on tile (0,0) for A + 3 on (64,64) for B
          into one psum (128, 126); rhs = pre-transposed x slices
    bias+evac: one DVE tensor_scalar_add -> outQ bf16 ((s,o), t layout)
  out DMA: (s,o),t layout direct to HBM; host transposes (O,T)->(T,O).

x is pre-transposed to (BN, C, L) bf16 on host; outputs returned bf16
and upcast on host. No PE/DVE/DMA transposes anywhere on device.
"""

import numpy as np
import ml_dtypes

import concourse.mybir as mybir
import concourse.bacc as bacc
from concourse.tile import TileContext
from concourse.bass_utils import run_bass_kernel_spmd

BF16 = mybir.dt.bfloat16
F32 = mybir.dt.float32

B = 32
N = 207
BN = B * N            # 6624
L = 128
C = 64                # in channels
O = 64                # out channels
KK = 3
META = 32
MA = META + 1         # augmented (bias row / ones col)
LOUT = L - KK + 1     # 126
NCORES = 8
PER = BN // NCORES    # 828
NTS = 128             # samples per tile
GX = 8                # samples per x-load / out-store DMA (4 pairs)


def build_program(per=PER):
    nc = bacc.Bacc("TRN2", target_bir_lowering=False)

    xT_d = nc.dram_tensor("xT", (per, C, L), BF16, kind="ExternalInput")
    metaT_d = nc.dram_tensor("metaT", (MA, per), BF16, kind="ExternalInput")
    w1T3_d = nc.dram_tensor("w1T3", (MA, C * MA), BF16, kind="ExternalInput")
    w2aug_d = nc.dram_tensor("w2aug", (MA, O * KK), BF16, kind="ExternalInput")
    blaug_d = nc.dram_tensor("blaug", (MA, O), BF16, kind="ExternalInput")
    out_d = nc.dram_tensor("out", (per, O, LOUT), BF16, kind="ExternalOutput")

    n_tiles = [(t, min(NTS, per - t)) for t in range(0, per, NTS)]

    with TileContext(nc) as tc:
        with (
            tc.tile_pool(name="const", bufs=1) as cpool,
            tc.tile_pool(name="wpool", bufs=2) as wpool,
            tc.tile_pool(name="xpool", bufs=3) as xpool,
            tc.tile_pool(name="hpool", bufs=3) as hpool,
            tc.tile_pool(name="opool", bufs=3) as opool,
            tc.tile_pool(name="pspool", bufs=2, space="PSUM") as pspool,
            tc.tile_pool(name="pspool1", bufs=1, space="PSUM") as psbpool,
        ):
            w1T3 = cpool.tile([MA, C * MA], BF16)
            nc.sync.dma_start(w1T3[:, :], w1T3_d[:, :])
            w2aug = cpool.tile([MA, O * KK], BF16)
            nc.sync.dma_start(w2aug[:, :], w2aug_d[:, :])
            blaug = cpool.tile([MA, O], BF16)
            nc.sync.dma_start(blaug[:, :], blaug_d[:, :])

            for n0, nt in n_tiles:
                nh = nt // 2
                metaT_sb = wpool.tile([MA, NTS], BF16, tag="meta")
                nc.sync.dma_start(metaT_sb[:, 0:nt], metaT_d[:, n0 : n0 + nt])
                metaT_r = metaT_sb[:, 0:nt].rearrange("p (h s) -> p s h", s=2)

                # ---- step1: W1T[e, (c, n)] via per-channel matmuls ----
                W1T = wpool.tile([MA, C * NTS], BF16, tag="w1t")
                for c4 in range(C // 4):
                    ps1 = psbpool.tile([MA, 512], F32, tag="ps1")
                    for k in range(4):
                        c = c4 * 4 + k
                        nc.tensor.matmul(
                            ps1[:, k * 128 : k * 128 + nt],
                            w1T3[:, c * MA : (c + 1) * MA],
                            metaT_sb[:, 0:nt],
                            start=True, stop=True,
                        )
                    if nt == NTS:
                        nc.scalar.copy(W1T[:, c4 * 512 : (c4 + 1) * 512], ps1[:, :])
                    else:
                        src = ps1[:, :].rearrange("p (k n) -> p k n", n=128)[
                            :, :, 0:nt
                        ]
                        dst = W1T[:, c4 * 512 : (c4 + 1) * 512].rearrange(
                            "p (c n) -> p c n", n=128
                        )[:, :, 0:nt]
                        nc.scalar.copy(dst, src)
                W1T_r = W1T[:, :].rearrange("p (c n) -> p n c", n=NTS)

                # ---- bias: B2[(s,o), pair] ----
                psB = psbpool.tile([128, 64], F32, tag="psB")
                nc.tensor.matmul(
                    psB[0:64, 0:nh], blaug[:, :], metaT_r[:, 0, :],
                    start=True, stop=True,
                )
                nc.tensor.matmul(
                    psB[64:128, 0:nh], blaug[:, :], metaT_r[:, 1, :],
                    start=True, stop=True,
                )
                B2 = wpool.tile([128, 64], F32, tag="B2")
                nc.vector.tensor_copy(B2[:, 0:nh], psB[:, 0:nh])

                # ---- per-pair stage ----
                for g0 in range(0, nt, GX):
                    gx = min(GX, nt - g0)
                    qn = gx // 2
                    xsb = xpool.tile([128, 4 * L], BF16, tag="xsb")
                    nc.sync.dma_start(
                        xsb[:, 0 : qn * L].rearrange("p (q l) -> p q l", l=L),
                        xT_d[n0 + g0 : n0 + g0 + gx, :, :].rearrange(
                            "(q s) c l -> (s c) q l", s=2
                        ),
                    )
                    outQ = opool.tile([128, 4 * LOUT], BF16, tag="outQ")
                    for q in range(qn):
                        nA = g0 + 2 * q
                        pidx = nA // 2
                        psH = pspool.tile([128, O * KK], F32, tag="psH")
                        nc.tensor.matmul(
                            psH[0:64, :], W1T_r[:, nA, :], w2aug[:, :],
                            start=True, stop=True,
                        )
                        nc.tensor.matmul(
                            psH[64:128, :], W1T_r[:, nA + 1, :], w2aug[:, :],
                            start=True, stop=True,
                        )
                        H2 = hpool.tile([128, O * KK], BF16, tag="H2")
                        nc.scalar.copy(H2[:, :], psH[:, :])
                        H2r = H2[:, :].rearrange("p (o j) -> p j o", j=KK)
                        # A and B accumulate in different PSUM banks (same-bank
                        # writes from different row tiles corrupt each other).
                        psCA = pspool.tile([128, LOUT], F32, tag="psCA")
                        psCB = pspool.tile([128, LOUT], F32, tag="psCB")
                        for j in range(KK):
                            nc.tensor.matmul(
                                psCA[0:64, :],
                                H2r[0:64, j, :],
                                xsb[0:64, q * L + j : q * L + j + LOUT],
                                start=(j == 0), stop=(j == KK - 1),
                            )
                            nc.tensor.matmul(
                                psCB[64:128, :],
                                H2r[64:128, j, :],
                                xsb[64:128, q * L + j : q * L + j + LOUT],
                                start=(j == 0), stop=(j == KK - 1),
                            )
                        nc.scalar.activation(
                            outQ[0:64, q * LOUT : (q + 1) * LOUT],
                            psCA[0:64, :],
                            mybir.ActivationFunctionType.Identity,
                            bias=B2[0:64, pidx : pidx + 1],
                        )
                        nc.vector.tensor_scalar_add(
                            outQ[64:128, q * LOUT : (q + 1) * LOUT],
                            psCB[64:128, :],
                            B2[64:128, pidx : pidx + 1],
                        )
                    nc.sync.dma_start(
                        out_d[n0 + g0 : n0 + g0 + gx, :, :].rearrange(
                            "(q s) o t -> (s o) q t", s=2
                        ),
                        outQ[:, 0 : qn * LOUT].rearrange("p (q t) -> p q t", t=LOUT),
                    )
    if not nc.is_finalized():
        nc.finalize()
    return nc


def _prep_consts(w1_w, w1_b, w2_w, w2_b, bl_w, bl_b, meta):
    bf = ml_dtypes.bfloat16
    w2aug = np.concatenate([w2_w.T, w2_b[None, :]], axis=0).astype(bf)
    blaug = np.concatenate([bl_w.T, bl_b[None, :]], axis=0).astype(bf)
    metaT = np.concatenate(
        [meta.T, np.ones((1, meta.shape[0]), np.float32)], axis=0
    ).astype(bf)
    w1r = w1_w.reshape(C, META, META)           # (c, e, m)
    w1T3 = np.zeros((C, MA, MA), np.float32)    # (c, m_aug, e_aug)
    w1T3[:, :META, :META] = w1r.transpose(0, 2, 1)
    w1T3[:, META, :META] = w1_b.reshape(C, META)
    w1T3[:, META, META] = 1.0
    w1T3 = np.ascontiguousarray(w1T3.transpose(1, 0, 2)).reshape(MA, C * MA).astype(bf)
    return w1T3, w2aug, blaug, metaT


LAST_EXEC_NS = None
_NC_CACHE = {}


def kernel(meta_knowledge, input, w1_w, w1_b, w2_w, w2_b, bl_w, bl_b):
    global LAST_EXEC_NS
    import os

    bf = ml_dtypes.bfloat16
    w1T3, w2aug, blaug, metaT = _prep_consts(
        w1_w, w1_b, w2_w, w2_b, bl_w, bl_b, meta_knowledge
    )
    xT = input.reshape(BN, L, C).transpose(0, 2, 1).astype(bf)  # (BN, C, L)

    if PER not in _NC_CACHE:
        _NC_CACHE[PER] = build_program(PER)
    nc = _NC_CACHE[PER]
    in_maps = []
    for i in range(NCORES):
        s = slice(i * PER, (i + 1) * PER)
        in_maps.append(
            {
                "xT": np.ascontiguousarray(xT[s]),
                "metaT": np.ascontiguousarray(metaT[:, s]),
                "w1T3": w1T3,
                "w2aug": w2aug,
                "blaug": blaug,
            }
        )
    trace = os.environ.get("KM_TRACE", "0") == "1"
    res = run_bass_kernel_spmd(
        nc, in_maps, core_ids=list(range(NCORES)), trace=trace
    )
    if res.exec_time_ns is not None:
        LAST_EXEC_NS = res.exec_time_ns
    out = np.concatenate([r["out"] for r in res.results], axis=0)  # (BN, O, LOUT) bf16
    out = out.astype(np.float32).transpose(0, 2, 1)
    return np.ascontiguousarray(out.reshape(B, N, LOUT, O))


# revision 7
# speedup vs baseline: 4.4805x; 1.2065x over previous
"""MetaConv1d Trainium2 kernel (v2 — transpose-free, tile-position paired).

Per-sample hypernetwork-generated conv1d:
  W1 = meta @ w1.T + b1            (BN, 64, 33)   [aug: col 32 generates ones row]
  H  = W1 @ w2.T + b2              (BN, 64, 192)  [192 = (o, j) = 3o+j]
  b  = meta @ bl.T + bb            (BN, 64)
  out[n,t,o] = sum_{c,j} H[n,c,3o+j] * x[n,c,t+j] + b[n,o]

Sharding: batch*node dim (6624) split evenly over 8 cores (828 each).

Device dataflow (per n-tile of <=128 samples):
  step1: per channel c: psum = w1T3_c.T @ metaT  -> W1T (33, 64*128) bf16
         (c even -> PE tile (0,0), c odd -> (0,64); no transposes needed
          because out partitions = e come straight from the matmul)
  bias:  2 MMs (even/odd samples) -> B2 (128, npairs) f32
  per pair of samples (A,B):
    step2: MM_A (tile 0,0) -> psH[0:64], MM_B (tile 0,64) -> psH[64:128]
    H2 evac: one scalar copy (128, 192) -> bf16
    conv: 3 accumulating MMs on tile (0,0) for A + 3 on (64,64) for B
          into one psum (128, 126); rhs = pre-transposed x slices
    bias+evac: one DVE tensor_scalar_add -> outQ bf16 ((s,o), t layout)
  out DMA: (s,o),t layout direct to HBM; host transposes (O,T)->(T,O).

x is pre-transposed to (BN, C, L) bf16 on host; outputs returned bf16
and upcast on host. No PE/DVE/DMA transposes anywhere on device.
"""

import numpy as np
import ml_dtypes

import concourse.mybir as mybir
import concourse.bacc as bacc
from concourse.tile import TileContext
from concourse.bass_utils import run_bass_kernel_spmd

BF16 = mybir.dt.bfloat16
F32 = mybir.dt.float32

B = 32
N = 207
BN = B * N            # 6624
L = 128
C = 64                # in channels
O = 64                # out channels
KK = 3
META = 32
MA = META + 1         # augmented (bias row / ones col)
LOUT = L - KK + 1     # 126
NCORES = 8
PER = BN // NCORES    # 828
NTS = 128             # samples per tile
GX = 8                # samples per x-load / out-store DMA (4 pairs)


def build_program(per=PER):
    nc = bacc.Bacc("TRN2", target_bir_lowering=False)

    xT_d = nc.dram_tensor("xT", (per, C, L), BF16, kind="ExternalInput")
    metaT_d = nc.dram_tensor("metaT", (MA, per), BF16, kind="ExternalInput")
    w1T3_d = nc.dram_tensor("w1T3", (MA, C * MA), BF16, kind="ExternalInput")
    w2aug_d = nc.dram_tensor("w2aug", (MA, O * KK), BF16, kind="ExternalInput")
    blaug_d = nc.dram_tensor("blaug", (MA, O), BF16, kind="ExternalInput")
    out_d = nc.dram_tensor("out", (per, O, LOUT), BF16, kind="ExternalOutput")

    n_tiles = [(t, min(NTS, per - t)) for t in range(0, per, NTS)]

    with TileContext(nc) as tc:
        with (
            tc.tile_pool(name="const", bufs=1) as cpool,
            tc.tile_pool(name="wpool", bufs=2) as wpool,
            tc.tile_pool(name="xpool", bufs=3) as xpool,
            tc.tile_pool(name="hpool", bufs=3) as hpool,
            tc.tile_pool(name="opool", bufs=3) as opool,
            tc.tile_pool(name="pspool", bufs=2, space="PSUM") as pspool,
            tc.tile_pool(name="pspool1", bufs=1, space="PSUM") as psbpool,
        ):
            w1T3 = cpool.tile([MA, C * MA], BF16)
            nc.sync.dma_start(w1T3[:, :], w1T3_d[:, :])
            w2aug = cpool.tile([MA, O * KK], BF16)
            nc.sync.dma_start(w2aug[:, :], w2aug_d[:, :])
            blaug = cpool.tile([MA, O], BF16)
            nc.sync.dma_start(blaug[:, :], blaug_d[:, :])

            for n0, nt in n_tiles:
                nh = nt // 2
                metaT_sb = wpool.tile([MA, NTS], BF16, tag="meta")
                nc.sync.dma_start(metaT_sb[:, 0:nt], metaT_d[:, n0 : n0 + nt])
                metaT_r = metaT_sb[:, 0:nt].rearrange("p (h s) -> p s h", s=2)

                # ---- step1: W1T[e, (c, n)] via per-channel matmuls ----
                W1T = wpool.tile([MA, C * NTS], BF16, tag="w1t")
                for c4 in range(C // 4):
                    ps1 = psbpool.tile([MA, 512], F32, tag="ps1")
                    for k in range(4):
                        c = c4 * 4 + k
                        nc.tensor.matmul(
                            ps1[:, k * 128 : k * 128 + nt],
                            w1T3[:, c * MA : (c + 1) * MA],
                            metaT_sb[:, 0:nt],
                            start=True, stop=True,
                        )
                    if nt == NTS:
                        nc.vector.tensor_copy(
                            W1T[:, c4 * 512 : (c4 + 1) * 512], ps1[:, :]
                        )
                    else:
                        src = ps1[:, :].rearrange("p (k n) -> p k n", n=128)[
                            :, :, 0:nt
                        ]
                        dst = W1T[:, c4 * 512 : (c4 + 1) * 512].rearrange(
                            "p (c n) -> p c n", n=128
                        )[:, :, 0:nt]
                        nc.vector.tensor_copy(dst, src)
                W1T_r = W1T[:, :].rearrange("p (c n) -> p n c", n=NTS)

                # ---- bias: B2[(s,o), pair] ----
                psB = psbpool.tile([128, 64], F32, tag="psB")
                nc.tensor.matmul(
                    psB[0:64, 0:nh], blaug[:, :], metaT_r[:, 0, :],
                    start=True, stop=True,
                )
                nc.tensor.matmul(
                    psB[64:128, 0:nh], blaug[:, :], metaT_r[:, 1, :],
                    start=True, stop=True,
                )
                B2 = wpool.tile([128, 64], F32, tag="B2")
                nc.vector.tensor_copy(B2[:, 0:nh], psB[:, 0:nh])

                # ---- per-pair stage ----
                for g0 in range(0, nt, GX):
                    gx = min(GX, nt - g0)
                    qn = gx // 2
                    xsb = xpool.tile([128, 4 * L], BF16, tag="xsb")
                    nc.sync.dma_start(
                        xsb[:, 0 : qn * L].rearrange("p (q l) -> p q l", l=L),
                        xT_d[n0 + g0 : n0 + g0 + gx, :, :].rearrange(
                            "(q s) c l -> (s c) q l", s=2
                        ),
                    )
                    outQ = opool.tile([128, 4 * LOUT], BF16, tag="outQ")
                    # hypernet step2 for 2 pairs per psum tile -> 1 evac copy
                    H2list = []
                    for q0 in range(0, qn, 2):
                        qq = min(2, qn - q0)
                        psH = pspool.tile([128, 2 * O * KK], F32, tag="psH")
                        for dq in range(qq):
                            nA = g0 + 2 * (q0 + dq)
                            hof = dq * O * KK
                            nc.tensor.matmul(
                                psH[0:64, hof : hof + O * KK],
                                W1T_r[:, nA, :], w2aug[:, :],
                                start=True, stop=True,
                            )
                            nc.tensor.matmul(
                                psH[64:128, hof : hof + O * KK],
                                W1T_r[:, nA + 1, :], w2aug[:, :],
                                start=True, stop=True,
                            )
                        H2 = hpool.tile([128, 2 * O * KK], BF16, tag="H2")
                        nc.scalar.copy(
                            H2[:, 0 : qq * O * KK], psH[:, 0 : qq * O * KK]
                        )
                        H2list.append(H2)
                    for q in range(qn):
                        nA = g0 + 2 * q
                        pidx = nA // 2
                        H2 = H2list[q // 2]
                        hof = (q % 2) * O * KK
                        H2r = H2[:, hof : hof + O * KK].rearrange(
                            "p (o j) -> p j o", j=KK
                        )
                        # A and B accumulate in different PSUM banks (same-bank
                        # writes from different row tiles corrupt each other).
                        psCA = pspool.tile([128, LOUT], F32, tag="psCA")
                        psCB = pspool.tile([128, LOUT], F32, tag="psCB")
                        for j in range(KK):
                            nc.tensor.matmul(
                                psCA[0:64, :],
                                H2r[0:64, j, :],
                                xsb[0:64, q * L + j : q * L + j + LOUT],
                                start=(j == 0), stop=(j == KK - 1),
                            )
                            nc.tensor.matmul(
                                psCB[64:128, :],
                                H2r[64:128, j, :],
                                xsb[64:128, q * L + j : q * L + j + LOUT],
                                start=(j == 0), stop=(j == KK - 1),
                            )
                        nc.scalar.activation(
                            outQ[0:64, q * LOUT : (q + 1) * LOUT],
                            psCA[0:64, :],
                            mybir.ActivationFunctionType.Identity,
                            bias=B2[0:64, pidx : pidx + 1],
                        )
                        nc.vector.tensor_scalar_add(
                            outQ[64:128, q * LOUT : (q + 1) * LOUT],
                            psCB[64:128, :],
                            B2[64:128, pidx : pidx + 1],
                        )
                    nc.sync.dma_start(
                        out_d[n0 + g0 : n0 + g0 + gx, :, :].rearrange(
                            "(q s) o t -> (s o) q t", s=2
                        ),
                        outQ[:, 0 : qn * LOUT].rearrange("p (q t) -> p q t", t=LOUT),
                    )
    if not nc.is_finalized():
        nc.finalize()
    return nc


def _prep_consts(w1_w, w1_b, w2_w, w2_b, bl_w, bl_b, meta):
    bf = ml_dtypes.bfloat16
    w2aug = np.concatenate([w2_w.T, w2_b[None, :]], axis=0).astype(bf)
    blaug = np.concatenate([bl_w.T, bl_b[None, :]], axis=0).astype(bf)
    metaT = np.concatenate(
        [meta.T, np.ones((1, meta.shape[0]), np.float32)], axis=0
    ).astype(bf)
    w1r = w1_w.reshape(C, META, META)           # (c, e, m)
    w1T3 = np.zeros((C, MA, MA), np.float32)    # (c, m_aug, e_aug)
    w1T3[:, :META, :META] = w1r.transpose(0, 2, 1)
    w1T3[:, META, :META] = w1_b.reshape(C, META)
    w1T3[:, META, META] = 1.0
    w1T3 = np.ascontiguousarray(w1T3.transpose(1, 0, 2)).reshape(MA, C * MA).astype(bf)
    return w1T3, w2aug, blaug, metaT


LAST_EXEC_NS = None
_NC_CACHE = {}


def kernel(meta_knowledge, input, w1_w, w1_b, w2_w, w2_b, bl_w, bl_b):
    global LAST_EXEC_NS
    import os

    bf = ml_dtypes.bfloat16
    w1T3, w2aug, blaug, metaT = _prep_consts(
        w1_w, w1_b, w2_w, w2_b, bl_w, bl_b, meta_knowledge
    )
    xT = input.reshape(BN, L, C).transpose(0, 2, 1).astype(bf)  # (BN, C, L)

    if PER not in _NC_CACHE:
        _NC_CACHE[PER] = build_program(PER)
    nc = _NC_CACHE[PER]
    in_maps = []
    for i in range(NCORES):
        s = slice(i * PER, (i + 1) * PER)
        in_maps.append(
            {
                "xT": np.ascontiguousarray(xT[s]),
                "metaT": np.ascontiguousarray(metaT[:, s]),
                "w1T3": w1T3,
                "w2aug": w2aug,
                "blaug": blaug,
            }
        )
    trace = os.environ.get("KM_TRACE", "0") == "1"
    res = run_bass_kernel_spmd(
        nc, in_maps, core_ids=list(range(NCORES)), trace=trace
    )
    if res.exec_time_ns is not None:
        LAST_EXEC_NS = res.exec_time_ns
    out = np.concatenate([r["out"] for r in res.results], axis=0)  # (BN, O, LOUT) bf16
    out = out.astype(np.float32).transpose(0, 2, 1)
    return np.ascontiguousarray(out.reshape(B, N, LOUT, O))
